# revision 6
# baseline (speedup 1.0000x reference)
"""Trainium2 Bass kernel for Kaldi LinearResample (16 kHz -> 22.05 kHz) on [8, 960000].

out[b, 441*(4q+m) + p] = sum_i x[b, 1280*q + 320*m - 6 + i] * B[i, p] with B the
[384, 441] polyphase tap matrix (13 taps/phase).  Pure data parallel: one batch
row per core.

v2 layout: the HOST pre-transposes the input into the exact lhsT layout the PE
needs (a strided numpy copy), so the device does NO transposes at all:

  XT[128j + u, 128c + ql] = xpad6[1280*(128j + ql) + 128c + u]   (bf16)

i.e. per q-tile j (128 consecutive q's), an SBUF tile [128, 11, 128] whose
partition dim is the sample-offset u within a 128-chunk c of the 1344-sample
window, and whose free dim is (chunk, q).  Per q-tile: one 2816B/partition DMA
in -> 14 accumulating bf16 matmuls against 7 unique shifted filter tiles
(packed to their structural column ranges, 965 cols total) into one 4-bank f32
PSUM tile (phase m at 512-aligned slot) -> PSUM->SBUF copy-cast to bf16 split
between ACT (phases 0,1) and DVE (phases 2,3) -> one 3528B/partition DMA out.

q is padded to 768 rows (6 uniform tiles of 128) so every DMA has a
multiple-of-16 partition count (single-SDMA-engine descriptor degeneration
otherwise); rows 750..767 compute zeros and are sliced off on the host.
Input rides the SP HWDGE ring, the filter rides the ACT ring, output stores
alternate between the gpsimd SWDGE path and the ACT ring.  Output is stored
bf16 (rel-err budget allows it) and widened to f32 on the host.
"""

import math

import numpy as np

N_IN = 960000
P_PH = 441
NQ = 750            # real q rows (4 blocks of 320 samples each)
NQPAD = 768         # padded to 6 uniform q-tiles of 128
NTILE = 6
NCH = 11            # 128-sample chunks per 1408-padded window
XPAD = 983168       # 1280*767 + 128*10 + 127 + 1
N_OUT = NQ * 4 * P_PH
N_CORES = 8

# per block-phase m: (chunk c, filter-tile shift off = 128c - 320m)
USE = {0: [(0, 0), (1, 128), (2, 256)],
       1: [(2, -64), (3, 64), (4, 192), (5, 320)],
       2: [(5, 0), (6, 128), (7, 256)],
       3: [(7, -64), (8, 64), (9, 192), (10, 320)]}
OFFS = [0, 128, 256, -64, 64, 192, 320]

_ORIG, _NEW, _LPW = 16000, 22050, 6


def _tables():
    """Packed filter [128, sum(widths)] bf16-able f32 + per-shift col ranges."""
    base = math.gcd(_ORIG, _NEW)
    P = _NEW // base
    cutoff = 0.99 * 0.5 * min(_ORIG, _NEW)
    ww = _LPW / (2.0 * cutoff)
    out_t = np.arange(P, dtype=np.float64) / _NEW
    min_i = np.ceil((out_t - ww) * _ORIG)
    max_i = np.floor((out_t + ww) * _ORIG)
    W = int((max_i - min_i + 1).max())
    j = np.arange(W, dtype=np.float64)
    inp_i = min_i[:, None] + j[None, :]
    dt = inp_i / _ORIG - out_t[:, None]
    w = np.zeros_like(dt)
    inside = np.abs(dt) < ww
    w[inside] = 0.5 * (1.0 + np.cos(2.0 * np.pi * cutoff / _LPW * dt[inside]))
    zero = dt == 0.0
    nz = ~zero
    w[nz] *= np.sin(2.0 * np.pi * cutoff * dt[nz]) / (np.pi * dt[nz])
    w[zero] *= 2.0 * cutoff
    w /= _ORIG
    fi = min_i.astype(np.int64)
    wf = w.astype(np.float32)
    Bfull = np.zeros((384, P), dtype=np.float32)
    for p in range(P):
        for jj in range(W):
            Bfull[fi[p] + 6 + jj, p] += wf[p, jj]
    lo = fi + 6
    colr, boff, packed = {}, {}, []
    pos = 0
    for off in OFFS:
        cols = np.where((lo + W - 1 >= off) & (lo <= off + 127))[0]
        c0, c1 = int(cols.min()), int(cols.max()) + 1
        colr[off] = (c0, c1)
        boff[off] = pos
        t = np.zeros((128, c1 - c0), dtype=np.float32)
        for r in range(128):
            src = off + r
            if 0 <= src < 384:
                t[r] = Bfull[src, c0:c1]
        packed.append(t)
        pos += c1 - c0
    return np.concatenate(packed, axis=1), colr, boff


_COLR: dict = {}
_BOFF: dict = {}
_BW = 0
_CACHE: dict = {}


def _build():
    if "nc" in _CACHE:
        return _CACHE["nc"]

    import concourse.bass as bass
    import concourse.tile as tile
    from concourse import bacc, mybir

    F32 = mybir.dt.float32
    BF16 = mybir.dt.bfloat16

    bw = _BW

    nc = bacc.Bacc("TRN2", target_bir_lowering=False, debug=False,
                   num_devices=N_CORES)
    x_dram = nc.declare_dram_parameter("xt", [NQPAD * NCH * 128], BF16,
                                       isOutput=False)
    b_dram = nc.declare_dram_parameter("bfilt", [128, bw], BF16, isOutput=False)
    o_dram = nc.declare_dram_parameter("out", [NQPAD * 4 * P_PH], BF16,
                                       isOutput=True)
    xh = x_dram.ap().tensor
    oh = o_dram.ap().tensor

    with tile.TileContext(nc) as tc:
        with (
            tc.tile_pool(name="sb", bufs=1) as spool,
            tc.tile_pool(name="pacc", bufs=2, space="PSUM") as paccpool,
        ):
            # A tiny dummy ACT op first: the compiler inserts the activation
            # table load right before the first ACTIVATE on the scalar
            # stream; anchored here it runs during the input stream instead
            # of delaying the first real PSUM->SBUF copy by 1.4us.
            scratch = spool.tile([128, 128], BF16)
            nc.vector.memset(scratch[:], 0.0)
            warmsb = spool.tile([128, 128], BF16)
            nc.scalar.mul(warmsb[:], scratch[:], 1.0)

            # bfilt FIRST, on the same ring as the even inputs: the 16 SDMA
            # engines drain each ring in FIFO order, so in-ring order =
            # completion priority.  Inputs split across BOTH HWDGE rings
            # (SP even tiles, ACT odd tiles) for more in-flight descriptors.
            bsb = spool.tile([128, bw], BF16)
            nc.sync.dma_start(bsb[:], b_dram[:, :])
            xts = []
            for j in range(NTILE):
                xt = spool.tile([128, NCH, 128], BF16, name=f"xt{j}")
                eng = nc.sync if j % 2 == 0 else nc.scalar
                eng.dma_start(
                    xt[:],
                    bass.AP(xh, NCH * 128 * 128 * j,
                            [[NCH * 128, 128], [1, NCH * 128]]),
                )
                xts.append(xt)

            # PE pre-warm: the HAM clock gate keeps PE at 1.2 GHz until it
            # has been busy for a full ~3.4us activity window.  Burn that
            # window on dummy matmuls while the first input DMA is still in
            # flight, so the real stream runs at 2.4 GHz almost from the
            # start.  The throwaway PSUM tile just rotates the pool once.
            warm = paccpool.tile([128, 4, 512], F32, name="warm", tag="pacc")
            for i in range(24):
                nc.tensor.matmul(warm[:, 0, 0:128], scratch[:], scratch[:],
                                 start=True, stop=True, skip_group_check=True)

            for j in range(NTILE):
                xt = xts[j]
                pacc = paccpool.tile([128, 4, 512], F32, name=f"pacc{j}", tag="pacc")
                for m in range(4):
                    uses = USE[m]
                    for ui, (c, off) in enumerate(uses):
                        c0, c1 = _COLR[off]
                        nc.tensor.matmul(
                            pacc[:, m, c0:c1],
                            xt[:, c, :],
                            bsb[:, _BOFF[off]:_BOFF[off] + (c1 - c0)],
                            start=(ui == 0),
                            stop=(ui == len(uses) - 1),
                        )

                # independent halves so the ACT and DVE copies never
                # serialize on a shared destination tile
                ota = spool.tile([128, 2, P_PH], BF16, name=f"ota{j}",
                                 bufs=3, tag="ota")
                otb = spool.tile([128, 2, P_PH], BF16, name=f"otb{j}",
                                 bufs=3, tag="otb")
                nc.scalar.mul(ota[:], pacc[:, 0:2, 0:P_PH], 1.0)
                nc.vector.tensor_copy(otb[:], pacc[:, 2:4, 0:P_PH])

                # stores ride the gpsimd SWDGE path: ACT stays free for the
                # PSUM->SBUF copies, SP stays free for the input ring
                nc.gpsimd.dma_start(
                    bass.AP(oh, 4 * P_PH * 128 * j,
                            [[4 * P_PH, 128], [1, 2 * P_PH]]),
                    ota[:],
                )
                nc.gpsimd.dma_start(
                    bass.AP(oh, 4 * P_PH * 128 * j + 2 * P_PH,
                            [[4 * P_PH, 128], [1, 2 * P_PH]]),
                    otb[:],
                )

    nc.compile()
    _CACHE["nc"] = nc
    return nc


def _prep():
    import ml_dtypes

    if "bmat" not in _CACHE:
        global _BW
        bmat, colr, boff = _tables()
        _COLR.update(colr)
        _BOFF.update(boff)
        _BW = bmat.shape[1]
        _CACHE["bmat"] = bmat.astype(ml_dtypes.bfloat16)
    return _CACHE["bmat"]


def _make_xt(x: np.ndarray) -> np.ndarray:
    """[768*1408] bf16: XT[128j+u, 128c+ql] = xpad6[1280*(128j+ql) + 128c + u]."""
    import ml_dtypes

    xpad = np.zeros(XPAD, dtype=ml_dtypes.bfloat16)
    xpad[6:6 + N_IN] = x.astype(ml_dtypes.bfloat16)
    v = np.lib.stride_tricks.as_strided(
        xpad, shape=(NTILE, 128, NCH, 128),
        strides=(2 * 1280 * 128, 2, 2 * 128, 2 * 1280))
    return np.ascontiguousarray(v).reshape(-1)


def _run(waveforms: np.ndarray, trace: bool = False):
    from concourse.bass_utils import run_bass_kernel_spmd

    bmat = _prep()
    nc = _build()
    in_maps = [
        {"xt": _make_xt(np.ascontiguousarray(waveforms[b], dtype=np.float32)),
         "bfilt": bmat}
        for b in range(N_CORES)
    ]
    last_err = None
    for attempt in range(3):
        try:
            res = run_bass_kernel_spmd(nc, in_maps, list(range(N_CORES)),
                                       trace=trace)
            out = np.stack([
                np.asarray(res.results[b]["out"]).reshape(NQPAD, 4 * P_PH)
                [:NQ].reshape(N_OUT)
                for b in range(N_CORES)
            ]).astype(np.float32)
            if not np.isfinite(out).all():
                raise RuntimeError("non-finite output (transient device "
                                   "corruption); retrying")
            return out, res
        except Exception as e:  # transient NRT device faults recover on retry
            last_err = e
            import time
            time.sleep(10)
    raise last_err


def kernel(waveforms: np.ndarray) -> np.ndarray:
    out, _ = _run(np.asarray(waveforms))
    return out
